# revision 17
# baseline (speedup 1.0000x reference)
"""Two-layer GAT (heads=1) + linear head on 8 Trainium2 NeuronCores.

Strategy: edges sorted by dst and sharded by dst-node range (edge-parallel per
the sharding hint, with shard cuts on 128-node block boundaries so all segment
reductions are core-local). Feature tables are built sharded and AllGathered.
Per-edge h[src] rows are fetched with the dma_gather custom DMA (int16 indices,
so the node table is split in 4 chunks of 25088 rows). Segment softmax + the
scatter-add aggregation are expressed as per-block one-hot matmuls accumulated
in PSUM; rare bucket overflow goes through a dma_scatter_add (CCE-add) path.
"""
import sys
import numpy as np

sys.path.insert(0, '/opt/trn_rl_repo')

import concourse.bass as bass
import concourse.bacc as bacc
import concourse.mybir as mybir
import concourse.tile as tile
from concourse.bass_utils import run_bass_kernel_spmd

F32 = mybir.dt.float32
I16 = mybir.dt.int16

LAST_EXEC_NS = None

NEG = 0.2
EPS = 1e-20


class GATConfig:
    def __init__(self, n_nodes, n_blocks_per_core, in_f=128, hid=48, out_f=32,
                 n_cores=8, spill_slices=3):
        self.N = n_nodes                    # real node count
        self.C = n_cores
        self.NB = n_blocks_per_core         # 128-node blocks per core
        self.PB = 128
        self.SH = self.NB * self.PB         # nodes per core (padded)
        self.NP = self.C * self.SH          # padded total nodes
        self.NCH = 4                        # idx chunks (int16 limit)
        assert self.NP % self.NCH == 0
        self.CHR = self.NP // self.NCH      # rows per chunk
        assert self.CHR <= 32767
        self.SPB = 4                        # slices per (block, chunk) bucket
        self.SLB = self.SPB * self.NCH      # slices per block (16)
        self.EB = self.SLB * 128            # edge slots per block (2048)
        self.BPC = 8                        # blocks per gather call
        self.IN = in_f
        self.HID = hid
        self.OUT = out_f
        self.ROW = 64                       # table row f32 elems (256B)
        self.SPS = spill_slices             # spill slices per (sc, dc) combo
        self.NSP = self.NCH * self.NCH * self.SPS   # spill slices per layer
        # gather-call schedule: list of (first_block, n_blocks)
        self.calls = []
        b = 0
        while b < self.NB:
            nb = min(self.BPC, self.NB - b)
            self.calls.append((b, nb))
            b += nb


def _wrap16(flat):
    """int16 idx list [M] -> [128, M/16] wrapped (pos i -> [i%16, i//16])."""
    M = flat.shape[0]
    assert M % 16 == 0
    w = flat.reshape(M // 16, 16).T.astype(np.int16)
    return np.tile(w, (8, 1))


def _host_prep_edges(cfg, src, dst):
    """Build all per-core edge-pass input arrays + the alpha slot map."""
    C, NB, NCH, SPB, SPS = cfg.C, cfg.NB, cfg.NCH, cfg.SPB, cfg.SPS
    E = src.shape[0]
    core = dst // cfg.SH
    block = (dst % cfg.SH) // 128
    dstloc = dst % 128
    chunk = src // cfg.CHR
    key = ((core * NB + block) * NCH + chunk)
    order = np.lexsort((src, key))
    k_s, src_s, dst_s = key[order], src[order], dst[order]
    counts = np.bincount(k_s, minlength=C * NB * NCH)
    starts = np.concatenate([[0], np.cumsum(counts)[:-1]])
    rank = np.arange(E) - starts[k_s]
    CAP = SPB * 128
    main = rank < CAP

    core_s = k_s // (NB * NCH)
    block_s = (k_s // NCH) % NB
    chunk_s = k_s % NCH
    dstloc_s = dst_s % 128

    # main-slot coordinates
    col16 = chunk_s * SPB + rank // 128          # slice within block (0..15)
    lane = rank % 128
    gpos = block_s * CAP + rank                  # pos within (core, chunk) region

    CHTOT = NB * CAP                             # idx slots per chunk region
    eidx = np.zeros((C, NCH, CHTOT), np.int16)
    dl = np.full((C, 128, NB * cfg.SLB), 999.0, np.float32)
    m = main
    eidx[core_s[m], chunk_s[m], gpos[m]] = (src_s[m] - chunk_s[m] * cfg.CHR).astype(np.int16)
    dl[core_s[m], lane[m], block_s[m] * cfg.SLB + col16[m]] = dstloc_s[m]

    # alpha slot map (position in the per-core ex2 buffers), for sorted order
    slot_core = core_s
    slot_main = m
    slot_col = np.where(m, block_s * cfg.SLB + col16, 0)
    slot_lane = np.where(m, lane, 0)

    # ---- spill ----
    sp_src = np.zeros((C, NCH, NCH * SPS * 128), np.int16)   # per (core, sc): (dc, s, lane)
    sp_dst = np.zeros((C, NCH, NCH * SPS * 128), np.int16)   # per (core, dc): (sc, s, lane)
    sp_sct = np.full((C, cfg.NSP * 128), cfg.SH, np.int16)   # scatter rows, default trash
    sp_of = ~main
    sp_idx = np.where(sp_of)[0]
    max_need = 0
    for ci in range(C):
        for sc in range(NCH):
            for dc in range(NCH):
                sel = sp_idx[(core_s[sp_idx] == ci) & (chunk_s[sp_idx] == sc)
                             & (dst_s[sp_idx] // cfg.CHR == dc)]
                if sel.size == 0:
                    continue
                # unique-dst per 128-lane slice: group by dup rank, split rank
                # groups at slice boundaries
                d = dst_s[sel]
                o2 = np.argsort(d, kind='stable')
                sel = sel[o2]
                d = d[o2]
                first = np.concatenate([[True], d[1:] != d[:-1]])
                gstart = np.zeros(d.shape[0], np.int64)
                gstart[first] = np.arange(d.shape[0])[first]
                gstart = np.maximum.accumulate(gstart)
                dup = np.arange(d.shape[0]) - gstart
                # greedy: first slice with room that doesn't already hold
                # this dst (exact per-slice dst sets; groups are small)
                s_assign = np.empty(d.shape[0], np.int64)
                l_assign = np.empty(d.shape[0], np.int64)
                fill, dsets = [], []
                order3 = np.lexsort((d, dup))
                for ei in order3:
                    dv = d[ei]
                    si = 0
                    while si < len(fill) and (fill[si] >= 128 or dv in dsets[si]):
                        si += 1
                    if si >= len(fill):
                        fill.append(0)
                        dsets.append(set())
                    s_assign[ei] = si
                    l_assign[ei] = fill[si]
                    fill[si] += 1
                    dsets[si].add(dv)
                snext = len(fill)
                max_need = max(max_need, snext)
                if snext > SPS:
                    raise RuntimeError(f"spill overflow: need {snext} > {SPS}")
                e_sel = sel
                s_a, l_a = s_assign, l_assign
                # src staging: ordered (sc, dc, s): pos = (dc*SPS+s)*128+lane
                ps = (dc * SPS + s_a) * 128 + l_a
                sp_src[ci, sc, ps] = (src_s[e_sel] - sc * cfg.CHR).astype(np.int16)
                # dst staging: ordered (dc, sc, s): pos = (sc*SPS+s)*128+lane
                pd = (sc * SPS + s_a) * 128 + l_a
                sp_dst[ci, dc, pd] = (dst_s[e_sel] - dc * cfg.CHR).astype(np.int16)
                # scatter idx + ex slot: global slice t = (sc*NCH+dc)*SPS+s
                t = (sc * NCH + dc) * SPS + s_a
                sp_sct[ci, t * 128 + l_a] = (dst_s[e_sel] % cfg.SH).astype(np.int16)
                slot_col[e_sel] = t
                slot_lane[e_sel] = l_a

    # wrapped versions
    eidx_w = np.zeros((C, NCH, 128, CHTOT // 16), np.int16)
    spsrc_w = np.zeros((C, NCH, 128, NCH * SPS * 8), np.int16)
    spdst_w = np.zeros((C, NCH, 128, NCH * SPS * 8), np.int16)
    spsct_w = np.zeros((C, 128, cfg.NSP * 8), np.int16)
    for ci in range(C):
        for ch in range(NCH):
            eidx_w[ci, ch] = _wrap16(eidx[ci, ch])
            spsrc_w[ci, ch] = _wrap16(sp_src[ci, ch])
            spdst_w[ci, ch] = _wrap16(sp_dst[ci, ch])
        spsct_w[ci] = _wrap16(sp_sct[ci])

    slot = dict(order=order, core=slot_core, main=slot_main,
                col=slot_col, lane=slot_lane, dst_sorted=dst_s)
    return eidx_w, dl, spsrc_w, spdst_w, spsct_w, slot, max_need


def _build_program(cfg):
    """Emit the full SPMD bass program. Returns (nc, names of IO)."""
    import os
    STAGE = int(os.environ.get("GAT_STAGE", "9"))
    C, NB, NCH, SPS, ROW = cfg.C, cfg.NB, cfg.NCH, cfg.SPS, cfg.ROW
    SH, NP, CHR, SLB, SPB = cfg.SH, cfg.NP, cfg.CHR, cfg.SLB, cfg.SPB
    HID, OUT = cfg.HID, cfg.OUT
    CHTOT = NB * SPB * 128

    nc = bacc.Bacc("TRN2", target_bir_lowering=False, debug=False,
                   num_devices=C, num_swdge_queues=4)

    # ---- inputs ----
    xT = nc.dram_tensor("xT", [128, SH], F32, kind="ExternalInput")
    W1e = nc.dram_tensor("W1e", [128, ROW], F32, kind="ExternalInput")
    W2e = nc.dram_tensor("W2e", [HID, ROW], F32, kind="ExternalInput")
    Wl = nc.dram_tensor("Wl", [OUT, OUT], F32, kind="ExternalInput")
    b1r = nc.dram_tensor("b1r", [128, HID], F32, kind="ExternalInput")
    b2r = nc.dram_tensor("b2r", [128, OUT], F32, kind="ExternalInput")
    blr = nc.dram_tensor("blr", [128, OUT], F32, kind="ExternalInput")
    iota = nc.dram_tensor("iota", [128, 128], F32, kind="ExternalInput")
    ident = nc.dram_tensor("ident", [128, 128], F32, kind="ExternalInput")
    ones_row = nc.dram_tensor("ones_row", [1, 128], F32, kind="ExternalInput")
    e_in, dl_in, sps_in, spd_in, spc_in = {}, {}, {}, {}, {}
    for L in (1, 2):
        for ch in range(NCH):
            e_in[L, ch] = nc.dram_tensor(f"e{L}c{ch}", [128, CHTOT // 16], I16,
                                         kind="ExternalInput")
            sps_in[L, ch] = nc.dram_tensor(f"sps{L}c{ch}", [128, NCH * SPS * 8],
                                           I16, kind="ExternalInput")
            spd_in[L, ch] = nc.dram_tensor(f"spd{L}c{ch}", [128, NCH * SPS * 8],
                                           I16, kind="ExternalInput")
        dl_in[L] = nc.dram_tensor(f"dl{L}", [128, NB * SLB], F32,
                                  kind="ExternalInput")
        spc_in[L] = nc.dram_tensor(f"spc{L}", [128, cfg.NSP * 8], I16,
                                   kind="ExternalInput")

    # ---- outputs ----
    mean_o = nc.dram_tensor("mean_o", [SH, OUT], F32, kind="ExternalOutput")
    s2_o = nc.dram_tensor("s2_o", [SH, 1], F32, kind="ExternalOutput")
    ex2_o = nc.dram_tensor("ex2_o", [128, NB * SLB], F32, kind="ExternalOutput")
    ex2sp_o = nc.dram_tensor("ex2sp_o", [128, cfg.NSP], F32, kind="ExternalOutput")

    # ---- internal DRAM ----
    T1s = nc.dram_tensor("T1s", [SH, ROW], F32)
    T2s = nc.dram_tensor("T2s", [SH, ROW], F32)
    T1f = nc.dram_tensor("T1f", [NP, ROW], F32, addr_space="Shared")
    T2f = nc.dram_tensor("T2f", [NP, ROW], F32, addr_space="Shared")
    Agg1 = nc.dram_tensor("Agg1", [SH + 128, ROW], F32)
    Agg2 = nc.dram_tensor("Agg2", [SH + 128, ROW], F32)
    ad1 = nc.dram_tensor("ad1", [SH, 1], F32)
    ad2 = nc.dram_tensor("ad2", [SH, 1], F32)

    with tile.TileContext(nc) as tc:
        with (
            tc.tile_pool(name="const", bufs=1) as cp,
            tc.tile_pool(name="x", bufs=3) as xp,
            tc.tile_pool(name="stage", bufs=2) as stp,
            tc.tile_pool(name="spill", bufs=1) as spp,
            tc.tile_pool(name="work", bufs=3) as wp,
            tc.tile_pool(name="mwork", bufs=4) as mp,
            tc.tile_pool(name="psA", bufs=2, space="PSUM") as psA,
            tc.tile_pool(name="psB", bufs=2, space="PSUM") as psB,
            tc.tile_pool(name="psC", bufs=2, space="PSUM") as psC,
        ):
            # persistent constants
            W1e_t = cp.tile([128, ROW], F32)
            nc.sync.dma_start(W1e_t[:], W1e[:])
            W2e_t = cp.tile([HID, ROW], F32)
            nc.sync.dma_start(W2e_t[:], W2e[:])
            Wl_t = cp.tile([OUT, OUT], F32)
            nc.sync.dma_start(Wl_t[:], Wl[:])
            b1_t = cp.tile([128, HID], F32)
            nc.sync.dma_start(b1_t[:], b1r[:])
            b2_t = cp.tile([128, OUT], F32)
            nc.sync.dma_start(b2_t[:], b2r[:])
            bl_t = cp.tile([128, OUT], F32)
            nc.sync.dma_start(bl_t[:], blr[:])
            io_t = cp.tile([128, 128], F32)
            nc.sync.dma_start(io_t[:], iota[:])
            id_t = cp.tile([128, 128], F32)
            nc.sync.dma_start(id_t[:], ident[:])
            on_t = cp.tile([1, 128], F32)
            nc.sync.dma_start(on_t[:], ones_row[:])

            # ================= phase 0: T1 shard =================
            SLAB = 8
            for s0 in range(0, NB, SLAB):
                ns = min(SLAB, NB - s0)
                xs = xp.tile([128, SLAB * 128], F32, tag="xslab")
                nc.sync.dma_start(xs[:, :ns * 128],
                                  xT[:, s0 * 128:(s0 + ns) * 128])
                slab = xp.tile([128, SLAB * ROW], F32, tag="t1slab")
                for t in range(ns):
                    pm = psA.tile([128, ROW], F32, tag="a")
                    nc.tensor.matmul(pm[:], lhsT=xs[:, t * 128:(t + 1) * 128],
                                     rhs=W1e_t[:], start=True, stop=True)
                    nc.vector.tensor_copy(slab[:, t * ROW:(t + 1) * ROW], pm[:])
                s3 = slab[:].rearrange("p (k f) -> p k f", f=ROW)
                nc.vector.memset(s3[:, :ns, 50:51], 1.0)
                nc.sync.dma_start(
                    T1s[s0 * 128:(s0 + ns) * 128, :].rearrange(
                        "(k p) f -> p k f", p=128),
                    s3[:, :ns, :])
                nc.sync.dma_start(
                    ad1[s0 * 128:(s0 + ns) * 128, :].rearrange(
                        "(k p) o -> p k o", p=128),
                    s3[:, :ns, 49:50])
            nc.gpsimd.collective_compute(
                "AllGather", mybir.AluOpType.bypass,
                replica_groups=[list(range(C))],
                ins=[T1s[:]], outs=[T1f[:]])

            # ================= per-layer edge pass =================
            def edge_pass(L, Tf, Agg, adloc, W, ex_out, ex_sp_out):
                # W = aggregation width (51 for L1, 35 for L2)
                AS_COL = W - 3   # asrc column (48 / 32)
                AD_COL = W - 2   # adst column (49 / 33)
                # main gathers: per chunk, calls of up to BPC blocks
                staged = {}   # (ch, call) -> tile
                for ci, (b0, nb) in enumerate(cfg.calls):
                    for ch in range(NCH):
                        nidx = nb * SPB * 128
                        st = stp.tile([128, cfg.BPC * SPB * ROW], F32,
                                      tag=f"stage{ch}")
                        it = mp.tile([128, cfg.BPC * SPB * 8], I16,
                                     tag=f"sidx{ch}")
                        c0 = b0 * SPB * 8
                        nc.sync.dma_start(it[:, :nidx // 16],
                                          e_in[L, ch][:, c0:c0 + nidx // 16])
                        nc.gpsimd.dma_gather(
                            out_ap=st[:, :nb * SPB * ROW].rearrange(
                                "p (n e) -> p n e", e=ROW),
                            in_ap=Tf[ch * CHR:(ch + 1) * CHR, :],
                            idxs_ap=it[:, :nidx // 16],
                            num_idxs=nidx, num_idxs_reg=nidx,
                            elem_size=ROW, queue_num=0, single_packet=False)
                        staged[ch, ci] = st

                    EP = int(os.environ.get("GAT_EP", "9"))
                    for bi in range(nb if EP >= 2 else 0):
                        b = b0 + bi
                        # block-level tiles
                        dlt = wp.tile([128, SLB], F32, tag="dl")
                        nc.sync.dma_start(dlt[:], dl_in[L][:, b * SLB:(b + 1) * SLB])
                        adr = wp.tile([1, 128], F32, tag="adrow")
                        nc.sync.dma_start(
                            adr[:],
                            adloc[b * 128:(b + 1) * 128, :].rearrange(
                                "(a p) o -> a (p o)", a=1))
                        prep = psB.tile([128, 128], F32, tag="b")
                        nc.tensor.matmul(prep[:], lhsT=on_t[:], rhs=adr[:],
                                         start=True, stop=True)
                        arep = wp.tile([128, 128], F32, tag="adrepS")
                        nc.vector.tensor_copy(arep[:], prep[:])

                        et = wp.tile([128, SLB], F32, tag="et")
                        junk = mp.tile([128, 128], F32, tag="junk")
                        for ch in range(NCH if EP >= 3 else 0):
                            st = staged[ch, ci]
                            base = bi * SPB * ROW
                            # adst expansion for the 4 slices of this bucket
                            for j in range(SPB):
                                s16 = ch * SPB + j
                                nc.vector.scalar_tensor_tensor(
                                    out=junk[:], in0=io_t[:],
                                    scalar=dlt[:, s16:s16 + 1], in1=arep[:],
                                    op0=mybir.AluOpType.is_equal,
                                    op1=mybir.AluOpType.mult,
                                    accum_out=et[:, s16:s16 + 1])
                            # e = asrc + adst for the bucket (strided asrc view)
                            asr = st[:].rearrange("p (n e) -> p n e", e=ROW)[
                                :, bi * SPB:(bi + 1) * SPB, AS_COL]
                            nc.vector.tensor_tensor(
                                out=et[:, ch * SPB:(ch + 1) * SPB],
                                in0=et[:, ch * SPB:(ch + 1) * SPB],
                                in1=asr, op=mybir.AluOpType.add)
                        # leaky relu + exp on the whole block
                        if EP < 3:
                            nc.vector.memset(et[:], 0.0)
                        lt = wp.tile([128, SLB], F32, tag="lt")
                        nc.vector.tensor_scalar_mul(lt[:], et[:], NEG)
                        nc.vector.tensor_max(et[:], et[:], lt[:])
                        xt_ = wp.tile([128, SLB], F32, tag="ext")
                        nc.scalar.activation(xt_[:], et[:],
                                             mybir.ActivationFunctionType.Exp)
                        if ex_out is not None:
                            nc.sync.dma_start(
                                ex_out[:, b * SLB:(b + 1) * SLB], xt_[:])
                        # aggregation matmuls
                        pagg = psC.tile([128, W], F32, tag="pagg")
                        if EP < 4:
                            nc.tensor.matmul(pagg[:], lhsT=io_t[:],
                                             rhs=st[:, :W], start=True, stop=True)
                        for s16 in range(SLB if EP >= 4 else 0):
                            ch = s16 // SPB
                            st = staged[ch, ci]
                            off = (bi * SPB + (s16 % SPB)) * ROW
                            Mt = mp.tile([128, 128], F32, tag="M")
                            nc.vector.tensor_scalar(
                                out=Mt[:], in0=io_t[:],
                                scalar1=dlt[:, s16:s16 + 1],
                                scalar2=xt_[:, s16:s16 + 1],
                                op0=mybir.AluOpType.is_equal,
                                op1=mybir.AluOpType.mult)
                            nc.tensor.matmul(pagg[:], lhsT=Mt[:],
                                             rhs=st[:, off:off + W],
                                             start=(s16 == 0), stop=(s16 == SLB - 1))
                        ag = wp.tile([128, W], F32, tag="aggS")
                        nc.vector.tensor_copy(ag[:], pagg[:])
                        nc.sync.dma_start(
                            Agg[b * 128:(b + 1) * 128, :W].rearrange(
                                "(a p) f -> p (a f)", a=1),
                            ag[:])

                # ---- spill ----
                import os as _os
                if int(_os.environ.get("GAT_SPILL", "1")) == 0:
                    return
                for sc in range(NCH):
                    for dc in range(NCH):
                        nsp = SPS * 128
                        st = spp.tile([128, SPS * ROW], F32, tag="spst")
                        it = mp.tile([128, SPS * 8], I16, tag="spsi")
                        nc.sync.dma_start(
                            it[:], sps_in[L, sc][:, dc * SPS * 8:(dc + 1) * SPS * 8])
                        nc.gpsimd.dma_gather(
                            out_ap=st[:].rearrange("p (n e) -> p n e", e=ROW),
                            in_ap=Tf[sc * CHR:(sc + 1) * CHR, :],
                            idxs_ap=it[:], num_idxs=nsp, num_idxs_reg=nsp,
                            elem_size=ROW, queue_num=0, single_packet=False)
                        dt_ = spp.tile([128, SPS * ROW], F32, tag="spdt")
                        it2 = mp.tile([128, SPS * 8], I16, tag="spdi")
                        nc.sync.dma_start(
                            it2[:], spd_in[L, dc][:, sc * SPS * 8:(sc + 1) * SPS * 8])
                        nc.gpsimd.dma_gather(
                            out_ap=dt_[:].rearrange("p (n e) -> p n e", e=ROW),
                            in_ap=Tf[dc * CHR:(dc + 1) * CHR, :],
                            idxs_ap=it2[:], num_idxs=nsp, num_idxs_reg=nsp,
                            elem_size=ROW, queue_num=0, single_packet=False)
                        for sp_s in range(SPS):
                            t = (sc * NCH + dc) * SPS + sp_s
                            xs_ = sp_s * ROW
                            xd_ = sp_s * ROW
                            ev = wp.tile([128, 1], F32, tag="spe")
                            nc.vector.tensor_tensor(
                                out=ev[:], in0=st[:, xs_ + AS_COL:xs_ + AS_COL + 1],
                                in1=dt_[:, xd_ + AD_COL:xd_ + AD_COL + 1],
                                op=mybir.AluOpType.add)
                            lv = wp.tile([128, 1], F32, tag="spl")
                            nc.vector.tensor_scalar_mul(lv[:], ev[:], NEG)
                            nc.vector.tensor_max(ev[:], ev[:], lv[:])
                            xv = wp.tile([128, 1], F32, tag="spx")
                            nc.scalar.activation(
                                xv[:], ev[:], mybir.ActivationFunctionType.Exp)
                            if ex_sp_out is not None:
                                nc.sync.dma_start(ex_sp_out[:, t:t + 1], xv[:])
                            vv = wp.tile([128, ROW], F32, tag="spv")
                            nc.vector.tensor_scalar_mul(
                                vv[:], st[:, xs_:xs_ + ROW], xv[:])
                            sit = mp.tile([128, 8], I16, tag="spsct")
                            nc.sync.dma_start(sit[:], spc_in[L][:, t * 8:(t + 1) * 8])
                            nc.gpsimd.dma_scatter_add(
                                out_ap=Agg[:],
                                in_ap=vv[:].rearrange("p (n e) -> p n e", e=ROW),
                                idxs_ap=sit[:], num_idxs=128, num_idxs_reg=128,
                                elem_size=ROW, queue_num=0, single_packet=False)

            # ================= layer 1 =================
            if STAGE >= 2:
                edge_pass(1, T1f, Agg1, ad1, 51, None, None)

            # layer-1 postprocess -> T2 shard
            for b in range(NB if STAGE >= 3 else 0):
                ag = wp.tile([128, 51], F32, tag="pp_ag")
                nc.sync.dma_start(
                    ag[:],
                    Agg1[b * 128:(b + 1) * 128, :51].rearrange(
                        "(a p) f -> p (a f)", a=1))
                sc_ = wp.tile([128, 1], F32, tag="pp_s")
                nc.vector.tensor_scalar_max(sc_[:], ag[:, 50:51], EPS)
                rc = wp.tile([128, 1], F32, tag="pp_r")
                nc.vector.reciprocal(rc[:], sc_[:])
                h = wp.tile([128, HID], F32, tag="pp_h")
                nc.vector.tensor_scalar_mul(h[:], ag[:, :HID], rc[:])
                nc.vector.tensor_add(h[:], h[:], b1_t[:])
                # ELU = max(x, min(exp(x)-1, 0))
                t1_ = wp.tile([128, HID], F32, tag="pp_t")
                nc.scalar.activation(t1_[:], h[:], mybir.ActivationFunctionType.Exp)
                nc.vector.tensor_scalar_add(t1_[:], t1_[:], -1.0)
                nc.vector.tensor_scalar_min(t1_[:], t1_[:], 0.0)
                nc.vector.tensor_max(h[:], h[:], t1_[:])
                # transpose then project: T2row = elu @ W2e
                pt = psB.tile([128, 128], F32, tag="b")
                nc.tensor.transpose(pt[:HID, :], h[:], id_t[:])
                hT = wp.tile([HID, 128], F32, tag="pp_hT")
                nc.vector.tensor_copy(hT[:], pt[:HID, :])
                pr = psA.tile([128, ROW], F32, tag="a")
                nc.tensor.matmul(pr[:], lhsT=hT[:], rhs=W2e_t[:],
                                 start=True, stop=True)
                r2 = wp.tile([128, ROW], F32, tag="pp_r2")
                nc.vector.tensor_copy(r2[:], pr[:])
                nc.vector.memset(r2[:, 34:35], 1.0)
                nc.sync.dma_start(
                    T2s[b * 128:(b + 1) * 128, :].rearrange(
                        "(a p) f -> p (a f)", a=1),
                    r2[:])
                nc.sync.dma_start(
                    ad2[b * 128:(b + 1) * 128, :].rearrange(
                        "(a p) o -> p (a o)", a=1),
                    r2[:, 33:34])
            if STAGE >= 4:
                nc.gpsimd.collective_compute(
                    "AllGather", mybir.AluOpType.bypass,
                    replica_groups=[list(range(C))],
                    ins=[T2s[:]], outs=[T2f[:]])

            # ================= layer 2 =================
            if STAGE >= 5:
                edge_pass(2, T2f, Agg2, ad2, 35, ex2_o, ex2sp_o)

            # layer-2 postprocess -> mean
            for b in range(NB if STAGE >= 6 else 0):
                ag = wp.tile([128, 35], F32, tag="q_ag")
                nc.sync.dma_start(
                    ag[:],
                    Agg2[b * 128:(b + 1) * 128, :35].rearrange(
                        "(a p) f -> p (a f)", a=1))
                sc_ = wp.tile([128, 1], F32, tag="q_s")
                nc.vector.tensor_scalar_max(sc_[:], ag[:, 34:35], EPS)
                nc.sync.dma_start(
                    s2_o[b * 128:(b + 1) * 128, :].rearrange(
                        "(a p) o -> p (a o)", a=1),
                    sc_[:])
                rc = wp.tile([128, 1], F32, tag="q_r")
                nc.vector.reciprocal(rc[:], sc_[:])
                h = wp.tile([128, OUT], F32, tag="q_h")
                nc.vector.tensor_scalar_mul(h[:], ag[:, :OUT], rc[:])
                nc.vector.tensor_add(h[:], h[:], b2_t[:])
                nc.scalar.activation(h[:], h[:], mybir.ActivationFunctionType.Relu)
                pt = psB.tile([128, 128], F32, tag="b")
                nc.tensor.transpose(pt[:OUT, :], h[:], id_t[:])
                hT = wp.tile([OUT, 128], F32, tag="q_hT")
                nc.vector.tensor_copy(hT[:], pt[:OUT, :])
                pm = psA.tile([128, ROW], F32, tag="a")
                nc.tensor.matmul(pm[:, :OUT], lhsT=hT[:], rhs=Wl_t[:],
                                 start=True, stop=True)
                mo = wp.tile([128, OUT], F32, tag="q_mo")
                nc.vector.tensor_copy(mo[:], pm[:, :OUT])
                nc.vector.tensor_add(mo[:], mo[:], bl_t[:])
                nc.sync.dma_start(
                    mean_o[b * 128:(b + 1) * 128, :].rearrange(
                        "(a p) f -> p (a f)", a=1),
                    mo[:])

    nc.compile()
    return nc


def _run(cfg, x, src, dst, W1, a_src1, a_dst1, b1, W2, a_src2, a_dst2, b2,
         Wl, bl, trace=False, sim=False):
    C = cfg.C
    while True:
        try:
            eidx_w, dl, spsrc_w, spdst_w, spsct_w, slot, _ = \
                _host_prep_edges(cfg, src, dst)
            break
        except RuntimeError:
            cfg.SPS += 2
            cfg.NSP = cfg.NCH * cfg.NCH * cfg.SPS

    # NOTE: layer 1 and layer 2 share the same edge structure
    nc = _build_program(cfg)

    xpad = np.zeros((cfg.NP, cfg.IN), np.float32)
    xpad[:cfg.N] = x
    W1e = np.zeros((cfg.IN, cfg.ROW), np.float32)
    W1e[:, :cfg.HID] = W1
    W1e[:, 48] = W1 @ a_src1
    W1e[:, 49] = W1 @ a_dst1
    W2e = np.zeros((cfg.HID, cfg.ROW), np.float32)
    W2e[:, :cfg.OUT] = W2
    W2e[:, 32] = W2 @ a_src2
    W2e[:, 33] = W2 @ a_dst2
    iota_np = np.tile(np.arange(128, dtype=np.float32)[None, :], (128, 1))

    in_maps = []
    for ci in range(C):
        m = dict(
            xT=np.ascontiguousarray(xpad[ci * cfg.SH:(ci + 1) * cfg.SH].T),
            W1e=W1e, W2e=W2e, Wl=Wl.astype(np.float32),
            b1r=np.tile(b1[None, :], (128, 1)).astype(np.float32),
            b2r=np.tile(b2[None, :], (128, 1)).astype(np.float32),
            blr=np.tile(bl[None, :], (128, 1)).astype(np.float32),
            iota=iota_np, ident=np.eye(128, dtype=np.float32),
            ones_row=np.ones((1, 128), np.float32),
        )
        for L in (1, 2):
            for ch in range(cfg.NCH):
                m[f"e{L}c{ch}"] = eidx_w[ci, ch]
                m[f"sps{L}c{ch}"] = spsrc_w[ci, ch]
                m[f"spd{L}c{ch}"] = spdst_w[ci, ch]
            m[f"dl{L}"] = dl[ci]
            m[f"spc{L}"] = spsct_w[ci]
        in_maps.append(m)

    if trace:
        import types
        try:
            import antenv.axon_hooks  # noqa: F401
        except ImportError:
            mod = types.ModuleType("antenv.axon_hooks")
            mod._hook = None
            def _set(h):
                mod._hook = h
            def _get():
                return mod._hook
            mod.set_axon_ntff_profile_hook = _set
            mod.get_axon_ntff_profile_hook = _get
            sys.modules["antenv.axon_hooks"] = mod
            from trn_agent_boot.trn_boot import _ntff_profile_via_ctypes
            _set(_ntff_profile_via_ctypes('/opt/axon/libaxon_pjrt.so'))
    if sim:
        from concourse import bass_interp
        msim = bass_interp.MultiCoreSim(nc, C)
        for ci in range(C):
            for k, v in in_maps[ci].items():
                msim.cores[ci].tensor(k)[:] = v
        msim.simulate()
        results = [{k: np.array(msim.cores[ci].tensor(k))
                    for k in ("mean_o", "s2_o", "ex2_o", "ex2sp_o")}
                   for ci in range(C)]

        class R:
            pass
        res = R()
        res.results = results
        res.exec_time_ns = None
    else:
        res = run_bass_kernel_spmd(nc, in_maps, list(range(C)), trace=trace)

    # ---- host assembly ----
    mean = np.concatenate([res.results[ci]["mean_o"] for ci in range(C)],
                          axis=0)[:cfg.N]
    s2 = np.concatenate([res.results[ci]["s2_o"][:, 0] for ci in range(C)])
    ex_main = np.stack([res.results[ci]["ex2_o"] for ci in range(C)])
    ex_sp = np.stack([res.results[ci]["ex2sp_o"] for ci in range(C)])
    co, mn = slot["core"], slot["main"]
    ln, cl = slot["lane"], slot["col"]
    exv = np.empty(co.shape[0], np.float32)
    exv[mn] = ex_main[co[mn], ln[mn], cl[mn]]
    exv[~mn] = ex_sp[co[~mn], ln[~mn], cl[~mn]]
    alpha_sorted = exv / s2[slot["dst_sorted"]]
    alpha = np.empty(src.shape[0], np.float32)
    alpha[slot["order"]] = alpha_sorted
    return mean.astype(np.float32), alpha.astype(np.float32), res


def kernel(x, edge_index, W1, a_src1, a_dst1, b1, W2, a_src2, a_dst2, b2,
           Wl, bl):
    import os
    global LAST_EXEC_NS
    x = np.asarray(x, np.float32)
    ei = np.asarray(edge_index)
    src = ei[0].astype(np.int64)
    dst = ei[1].astype(np.int64)
    cfg = GATConfig(n_nodes=x.shape[0], n_blocks_per_core=98)
    mean, alpha, _res = _run(cfg, x, src, dst,
                          np.asarray(W1, np.float32), np.asarray(a_src1, np.float32),
                          np.asarray(a_dst1, np.float32), np.asarray(b1, np.float32),
                          np.asarray(W2, np.float32), np.asarray(a_src2, np.float32),
                          np.asarray(a_dst2, np.float32), np.asarray(b2, np.float32),
                          np.asarray(Wl, np.float32), np.asarray(bl, np.float32),
                          trace=os.environ.get("GAT_TRACE", "0") == "1")
    LAST_EXEC_NS = getattr(_res, "exec_time_ns", None)
    return (mean, alpha)
